# revision 16
# baseline (speedup 1.0000x reference)
"""Fused multi-head self-attention (B=4, T=2048, C=1024, H=16) for 8 TRN2 NeuronCores.

Sharding: core c = (b, hg) with b = c // 2 (batch), hg = c % 2 (head group of 8
heads).  Each core computes its batch's QKV projection restricted to its 8 heads
(tensor-parallel split of the weight output dim) and the full attention for those
(batch, head) pairs.  No cross-core communication; outputs are disjoint slices of
the final [B, T, C] tensor.

Per-core kernel (see emit()):
  - x and W are loaded natural (contiguous DMA), transposed on the PE (fp32
    transpose tiles into PSUM) and cast-copied to fp16 SBUF alternately by the
    Vector and Scalar engines — an engine-only path with no DMA ordering hazards.
  - fp16 operands run the PE at 1 elem/cycle/lane; accumulation stays fp32 PSUM.
  - qT/kT per head-pair [128 (2 heads x 64 dims), T] = W_pair @ xT.
  - V is projected per (s_chunk, head-pair) into v_aug [128, SC, 8, 65] (fp16)
    whose ones 65th column makes the P @ V_aug matmul also emit softmax row-sums.
  - scores^T panels [s 128, 2 heads, t 512] = kT_chunk.T @ qT with the two heads
    packed in the PE array via row tiling (rows 0-63 / 64-127, concurrent).
  - exp on ScalarE (1/sqrt(64) folded into the activation scale), PSUM -> SBUF
    fp16, one N=1024 call per head-pair panel.
  - O_aug^T [65, 512] += v_aug_chunk.T @ P^T accumulated over s-chunks in PSUM.
  - epilogue: PSUM -> SBUF fp16 copy, xbar DMA-transpose (SBUF->SBUF) back to the
    natural [t, d] layout, reciprocal + per-partition scale on VectorE,
    contiguous DMA out per t-block.

Scheduling: the engines execute in instruction order, and the scalar engine's
exp stream is the bound, so all PE work that is not a scores matmul is either
done in a minimal pre-phase (W row-chunk 0 + x transposes + pair-0 projection,
with the first two score panels emitted as early as deps allow) or queued and
pumped one small item per s-loop period into the PE slack of the running
attention: remaining W row-chunks, later pairs' projections (quarter-groups) and
per-pair V projection chunks.
"""

from contextlib import ExitStack

import numpy as np

import concourse.bass as bass
import concourse.bacc as bacc
import concourse.tile as tile
from concourse import mybir
from concourse import bass_utils

F32 = mybir.dt.float32
F16 = mybir.dt.float16

B = 4
T = 2048
CIN = 1024
COUT = 512               # per-core output channels (8 heads x 64)
KC = CIN // 128          # contraction chunks
NPAIR = 4                # head pairs per core
D = 64
N_CORES = 8


def emit(ctx: ExitStack, tc: tile.TileContext, out_ap: bass.AP, ins: dict,
         T: int = T, mm_dt=F16, packed_scores: bool = True):
    nc = tc.nc
    x, wq, wk, wv, bq, bk, bv, ident = (
        ins[k] for k in ("x", "wq", "wk", "wv", "bq", "bk", "bv", "ident")
    )
    SC = T // 128            # s-chunks (also t-chunks)
    NB = T // 512            # 512-wide column blocks (proj N-chunks and t-blocks)

    # ---------------- pools ----------------
    consts = ctx.enter_context(tc.tile_pool(name="consts", bufs=1))
    cb_pool = ctx.enter_context(tc.tile_pool(name="cb_pool", bufs=3))
    wpool = ctx.enter_context(tc.tile_pool(name="wpool", bufs=1))
    xpool = ctx.enter_context(tc.tile_pool(name="xpool", bufs=1))
    vpool = ctx.enter_context(tc.tile_pool(name="vpool", bufs=1))
    qkpool = ctx.enter_context(tc.tile_pool(name="qkpool", bufs=2))
    ptpool = ctx.enter_context(tc.tile_pool(name="ptpool", bufs=5))
    osb_pool = ctx.enter_context(tc.tile_pool(name="osb_pool", bufs=2))
    trs_pool = ctx.enter_context(tc.tile_pool(name="trs_pool", bufs=4))
    rspool = ctx.enter_context(tc.tile_pool(name="rspool", bufs=4))
    outpool = ctx.enter_context(tc.tile_pool(name="outpool", bufs=2))

    psum_misc = ctx.enter_context(tc.tile_pool(name="psum_misc", bufs=2, space="PSUM"))
    psum_sring = ctx.enter_context(tc.tile_pool(name="psum_sring", bufs=2, space="PSUM"))
    psum_o = ctx.enter_context(tc.tile_pool(name="psum_o", bufs=2, space="PSUM"))

    # ---------------- constants ----------------
    identity = consts.tile([128, 128], F32)
    nc.sync.dma_start(out=identity, in_=ident)

    bq_sb = consts.tile([128, NPAIR], F32)
    bk_sb = consts.tile([128, NPAIR], F32)
    for p in range(NPAIR):
        nc.sync.dma_start(out=bq_sb[:, p : p + 1], in_=bq[p * 128 : (p + 1) * 128])
        nc.sync.dma_start(out=bk_sb[:, p : p + 1], in_=bk[p * 128 : (p + 1) * 128])
    bv_bcast = consts.tile([128, COUT], F32)
    nc.sync.dma_start(
        out=bv_bcast,
        in_=bass.AP(tensor=bv.tensor, offset=bv.offset, ap=[[0, 128]] + bv.ap),
    )

    # ---------------- transposed fp16 loads: PE transpose + cast-copies -------
    # dst layout [128 (c_in within chunk), KC, ncols]; copies alternate DVE/ACT.
    copy_flip = [0]
    cb_cache = {}

    def transpose_group(dst, src, r, g, nm):
        """Transpose kc group g (4 of the 8 contraction chunks) of row-chunk r."""
        key = (nm, r)
        if key not in cb_cache:
            cb = cb_pool.tile([128, CIN], F32, tag="cb", name=f"cb_{nm}_{r}")
            nc.sync.dma_start(out=cb, in_=src[r * 128 : (r + 1) * 128, :])
            cb_cache[key] = cb
        cb = cb_cache[key]
        trg = psum_misc.tile([128, 4, 128], F32, tag="proj", name=f"trg_{nm}_{r}_{g}")
        for k4 in range(4):
            kc = g * 4 + k4
            nc.tensor.transpose(
                trg[:, k4, :], cb[:, kc * 128 : (kc + 1) * 128], identity
            )
        if g == 1:
            cb_cache.pop(key)
        dst_sl = dst[:, g * 4 : (g + 1) * 4, r * 128 : (r + 1) * 128]
        if copy_flip[0] % 2 == 0:
            nc.vector.tensor_copy(dst_sl, trg)
        else:
            nc.scalar.copy(dst_sl, trg)
        copy_flip[0] += 1

    wq_t = wpool.tile([128, KC, COUT], mm_dt)
    wk_t = wpool.tile([128, KC, COUT], mm_dt)
    wv_t = wpool.tile([128, KC, COUT], mm_dt)
    x_t = xpool.tile([128, KC, T], mm_dt)

    # ---------------- projections ----------------
    proj_ps = {}

    def qk_proj_quarter(p, dst, wt, b_sb, nm, nb, quarter):
        """Quarter of a projection column-group: 2 contraction chunks; the last
        quarter adds the bias and writes fp16 SBUF."""
        if quarter == 0:
            ps = psum_misc.tile([128, 512], F32, tag="proj", name=f"ps_{nm}_{p}_{nb}")
            proj_ps[(nm, p, nb)] = ps
        ps = proj_ps[(nm, p, nb)]
        for k2 in range(2):
            kc = quarter * 2 + k2
            nc.tensor.matmul(
                ps,
                wt[:, kc, p * 128 : (p + 1) * 128],
                x_t[:, kc, nb * 512 : (nb + 1) * 512],
                start=(kc == 0),
                stop=(kc == KC - 1),
            )
        if quarter == 3:
            proj_ps.pop((nm, p, nb))
            nc.vector.tensor_scalar_add(
                dst[:, nb * 512 : (nb + 1) * 512], ps, b_sb[:, p : p + 1]
            )

    # v_aug [128 (s within chunk), SC, 8 heads, 65]; col 64 == 1.0
    v_aug = vpool.tile([128, SC, 8, 65], mm_dt)
    nc.vector.memset(v_aug[:, :, :, 64:65], 1.0)

    def v_chunk_pair(m, p):
        """Project V for s-chunk m, head pair p only (N=128)."""
        ps = psum_misc.tile([128, 128], F32, tag="proj", name=f"psv_{m}_{p}")
        for kc in range(KC):
            nc.tensor.matmul(
                ps,
                x_t[:, kc, m * 128 : (m + 1) * 128],
                wv_t[:, kc, p * 128 : (p + 1) * 128],
                start=(kc == 0),
                stop=(kc == KC - 1),
            )
        nc.vector.tensor_add(
            v_aug[:, m, 2 * p : 2 * p + 2, 0:64],
            ps.rearrange("a (h d) -> a h d", h=2),
            bv_bcast[:, p * 128 : (p + 1) * 128].rearrange("a (h d) -> a h d", h=2),
        )

    # ---------------- scores + exp ----------------
    def scores_exp(p, tb, sj, q_t, k_t):
        sl = psum_sring.tile([128, 2, 512], F32, tag="s", name=f"sl_{p}_{tb}_{sj}")
        for h in range(2):
            nc.tensor.matmul(
                sl[:, h, :],
                k_t[h * 64 : h * 64 + 64, sj * 128 : (sj + 1) * 128],
                q_t[h * 64 : h * 64 + 64, tb * 512 : (tb + 1) * 512],
                start=True,
                stop=True,
                tile_position=(h * 64, 0) if packed_scores else None,
            )
        pt = ptpool.tile([128, 2, 512], mm_dt, tag="pt", name=f"pt_{p}_{tb}_{sj}")
        nc.scalar.activation(pt, sl, mybir.ActivationFunctionType.Exp, scale=0.125)
        return pt

    # background work queue, pumped into the attention loop's PE slack
    bg = []

    def pump(n):
        for _ in range(n):
            if bg:
                bg.pop(0)()

    # ---------------- per head-pair attention ----------------
    def attention_pair(p, q_t, k_t, early_panels=None, interleave_v=False):
        early_panels = early_panels or {}
        out_stage = outpool.tile([128, SC, 128], F32, tag="ostage", name=f"ostage_{p}")
        for tb in range(NB):
            o_ps = [
                psum_o.tile([65, 512], F32, tag="o", name=f"o_{p}_{tb}_{h}")
                for h in range(2)
            ]
            for sj in range(SC):
                pt = early_panels.pop((tb, sj), None)
                if pt is None:
                    pt = scores_exp(p, tb, sj, q_t, k_t)
                if interleave_v and tb == 0:
                    if sj + 1 < SC:
                        v_chunk_pair(sj + 1, p)
                else:
                    pump(1)
                for h in range(2):
                    nc.tensor.matmul(
                        o_ps[h],
                        v_aug[:, sj, 2 * p + h, :],
                        pt[:, h, :],
                        start=(sj == 0),
                        stop=(sj == SC - 1),
                        skip_group_check=True,
                    )
            # epilogue: fp16 copy, xbar transpose to natural layout, normalize
            for h in range(2):
                o_sb = osb_pool.tile([80, 512], mm_dt, tag="osb", name=f"osb_{p}_{tb}_{h}")
                nc.gpsimd.memset(o_sb[64:80, :], 0.0)
                nc.vector.tensor_copy(o_sb[0:65, :], o_ps[h])
                for j in range(4):
                    tr_sb = trs_pool.tile([128, 80], mm_dt, tag="trs",
                                          name=f"trs_{p}_{tb}_{h}_{j}")
                    nc.sync.dma_start(
                        out=tr_sb, in_=o_sb[:, j * 128 : (j + 1) * 128], transpose=True
                    )
                    rs = rspool.tile([128, 1], F32, tag="rs", name=f"rs_{p}_{tb}_{h}_{j}")
                    nc.vector.reciprocal(rs, tr_sb[:, 64:65])
                    nc.vector.tensor_scalar_mul(
                        out_stage[:, tb * 4 + j, h * 64 : (h + 1) * 64],
                        tr_sb[:, 0:64],
                        rs,
                    )
            for j in range(tb * 4, tb * 4 + 4):
                nc.sync.dma_start(
                    out=out_ap[j * 128 : (j + 1) * 128, p * 128 : (p + 1) * 128],
                    in_=out_stage[:, j, :],
                )

    # ---------------- pre-phase: minimal work before the first exp ----------
    # W row-chunk 0 (pair-0 weights) + x transposes interleaved with pair-0
    # projection groups; the first two score panels are emitted as soon as
    # their dependencies exist (the PSUM score ring is 2 deep).
    for g in range(2):
        transpose_group(wq_t, wq, 0, g, "wq")
    for g in range(2):
        transpose_group(wk_t, wk, 0, g, "wk")
    q0 = qkpool.tile([128, T], mm_dt, tag="q", name="qT_0")
    k0 = qkpool.tile([128, T], mm_dt, tag="k", name="kT_0")
    early = {}
    for nb in range(NB):
        for m in range(nb * 4, nb * 4 + 4):
            for g in range(2):
                transpose_group(x_t, x, m, g, "x")
        for quarter in range(4):
            qk_proj_quarter(0, q0, wq_t, bq_sb, "q", nb, quarter)
        for quarter in range(4):
            qk_proj_quarter(0, k0, wk_t, bk_sb, "k", nb, quarter)
        if nb == 0:
            early[(0, 0)] = scores_exp(0, 0, 0, q0, k0)
            early[(0, 1)] = scores_exp(0, 0, 1, q0, k0)
            # wv row-chunk 0 (pair-0 V weights) right after the first panels
            for g in range(2):
                transpose_group(wv_t, wv, 0, g, "wv")
    # wv row-chunks for pairs 1-3 follow (needed before those pairs' V chunks)
    for r in range(1, COUT // 128):
        for g in range(2):
            transpose_group(wv_t, wv, r, g, "wv")
    # pair-0 V chunk 0 (chunks 1..15 interleave into tb0's s-loop)
    v_chunk_pair(0, 0)

    qk_tiles = {0: (q0, k0)}
    for p in range(NPAIR):
        if p + 1 < NPAIR:
            # queue pair p+1's W transposes, projections and V for the pump
            qn = qkpool.tile([128, T], mm_dt, tag="q", name=f"qT_{p+1}")
            kn = qkpool.tile([128, T], mm_dt, tag="k", name=f"kT_{p+1}")
            qk_tiles[p + 1] = (qn, kn)
            pn = p + 1
            for g in range(2):
                bg.append(lambda g=g, pn=pn: transpose_group(wq_t, wq, pn, g, "wq"))
            for g in range(2):
                bg.append(lambda g=g, pn=pn: transpose_group(wk_t, wk, pn, g, "wk"))
            for nb in range(NB):
                for quarter in range(4):
                    bg.append(lambda nb=nb, quarter=quarter, qn=qn, pn=pn:
                              qk_proj_quarter(pn, qn, wq_t, bq_sb, "q", nb, quarter))
                for quarter in range(4):
                    bg.append(lambda nb=nb, quarter=quarter, kn=kn, pn=pn:
                              qk_proj_quarter(pn, kn, wk_t, bk_sb, "k", nb, quarter))
            for m in range(SC):
                bg.append(lambda m=m, pn=pn: v_chunk_pair(m, pn))
        attention_pair(p, *qk_tiles[p], early_panels=(early if p == 0 else None),
                       interleave_v=(p == 0))
        while bg:
            bg.pop(0)()


def build_nc(T: int = T, mm_dt=F16, packed_scores: bool = True, num_devices: int = N_CORES):
    nc = bacc.Bacc("TRN2", target_bir_lowering=False, debug=False, num_devices=num_devices)
    ins = {
        "x": nc.dram_tensor("x", [T, CIN], F32, kind="ExternalInput").ap(),
        "wq": nc.dram_tensor("wq", [COUT, CIN], F32, kind="ExternalInput").ap(),
        "wk": nc.dram_tensor("wk", [COUT, CIN], F32, kind="ExternalInput").ap(),
        "wv": nc.dram_tensor("wv", [COUT, CIN], F32, kind="ExternalInput").ap(),
        "bq": nc.dram_tensor("bq", [COUT], F32, kind="ExternalInput").ap(),
        "bk": nc.dram_tensor("bk", [COUT], F32, kind="ExternalInput").ap(),
        "bv": nc.dram_tensor("bv", [COUT], F32, kind="ExternalInput").ap(),
        "ident": nc.dram_tensor("ident", [128, 128], F32, kind="ExternalInput").ap(),
    }
    out_ap = nc.dram_tensor("out", [T, COUT], F32, kind="ExternalOutput").ap()
    with tile.TileContext(nc) as tc:
        with ExitStack() as ctx:
            emit(ctx, tc, out_ap, ins, T=T, mm_dt=mm_dt, packed_scores=packed_scores)
    nc.compile()
    return nc


_NC = None
_IDENT = np.eye(128, dtype=np.float32)


def _get_nc():
    global _NC
    if _NC is None:
        _NC = build_nc()
    return _NC


def _make_in_maps(q_x, Wq, bq, Wk, bk, Wv, bv):
    f32 = lambda a: np.ascontiguousarray(np.asarray(a, dtype=np.float32))
    q_x, Wq, bq, Wk, bk, Wv, bv = map(f32, (q_x, Wq, bq, Wk, bk, Wv, bv))
    in_maps = []
    for c in range(N_CORES):
        b, hg = divmod(c, 2)
        sl = slice(hg * COUT, (hg + 1) * COUT)
        in_maps.append({
            "x": q_x[b],
            "wq": np.ascontiguousarray(Wq[sl]),
            "wk": np.ascontiguousarray(Wk[sl]),
            "wv": np.ascontiguousarray(Wv[sl]),
            "bq": np.ascontiguousarray(bq[sl]),
            "bk": np.ascontiguousarray(bk[sl]),
            "bv": np.ascontiguousarray(bv[sl]),
            "ident": _IDENT,
        })
    return in_maps


def kernel(q_x, Wq, bq, Wk, bk, Wv, bv):
    nc = _get_nc()
    in_maps = _make_in_maps(q_x, Wq, bq, Wk, bk, Wv, bv)
    res = bass_utils.run_bass_kernel_spmd(nc, in_maps, core_ids=list(range(N_CORES)))
    out = np.empty((B, T, CIN), np.float32)
    for c in range(N_CORES):
        b, hg = divmod(c, 2)
        out[b, :, hg * COUT : (hg + 1) * COUT] = res.results[c]["out"]
    return out


# revision 17
# speedup vs baseline: 1.0619x; 1.0619x over previous
"""Fused multi-head self-attention (B=4, T=2048, C=1024, H=16) for 8 TRN2 NeuronCores.

Sharding: core c = (b, hg) with b = c // 2 (batch), hg = c % 2 (head group of 8
heads).  Each core computes its batch's QKV projection restricted to its 8 heads
(tensor-parallel split of the weight output dim) and the full attention for those
(batch, head) pairs.  No cross-core communication; outputs are disjoint slices of
the final [B, T, C] tensor.

Per-core kernel (see emit()):
  - x and W are loaded natural (contiguous DMA), transposed on the PE (fp32
    transpose tiles into PSUM) and cast-copied to fp16 SBUF alternately by the
    Vector and Scalar engines — an engine-only path with no DMA ordering hazards.
  - fp16 operands run the PE at 1 elem/cycle/lane; accumulation stays fp32 PSUM.
  - qT/kT per head-pair [128 (2 heads x 64 dims), T] = W_pair @ xT; pair-0's
    projection groups are interleaved with the x-transpose stream and later
    pairs' projections are pumped into the PE slack of the previous pair's
    attention loop.
  - V stored per (s_chunk, head) as v_aug [128, SC, 8, 65] (fp16) with a ones
    65th column so the P @ V_aug matmul also emits the softmax row-sums; V
    projection chunks are split in half and interleaved into pair-0's first
    t-block s-loop.
  - scores^T panels [s 128, 2 heads, t 512] = kT_chunk.T @ qT with the two heads
    packed in the PE array via row tiling (rows 0-63 / 64-127, concurrent).
  - exp on ScalarE (1/sqrt(64) folded into the activation scale), PSUM -> SBUF
    fp16, one N=1024 call per head-pair panel.
  - O_aug^T [65, 512] += v_aug_chunk.T @ P^T accumulated over s-chunks in PSUM.
  - epilogue: PSUM -> SBUF fp16 copy, xbar DMA-transpose (SBUF->SBUF) back to the
    natural [t, d] layout, reciprocal + per-partition scale on VectorE,
    contiguous DMA out per t-block.
"""

from contextlib import ExitStack

import numpy as np

import concourse.bass as bass
import concourse.bacc as bacc
import concourse.tile as tile
from concourse import mybir
from concourse import bass_utils

F32 = mybir.dt.float32
F16 = mybir.dt.float16

B = 4
T = 2048
CIN = 1024
COUT = 512               # per-core output channels (8 heads x 64)
KC = CIN // 128          # contraction chunks
NPAIR = 4                # head pairs per core
D = 64
N_CORES = 8


def emit(ctx: ExitStack, tc: tile.TileContext, out_ap: bass.AP, ins: dict,
         T: int = T, mm_dt=F16, packed_scores: bool = True):
    nc = tc.nc
    x, wq, wk, wv, bq, bk, bv, ident = (
        ins[k] for k in ("x", "wq", "wk", "wv", "bq", "bk", "bv", "ident")
    )
    SC = T // 128            # s-chunks (also t-chunks)
    NB = T // 512            # 512-wide column blocks (proj N-chunks and t-blocks)

    # ---------------- pools ----------------
    consts = ctx.enter_context(tc.tile_pool(name="consts", bufs=1))
    cb_pool = ctx.enter_context(tc.tile_pool(name="cb_pool", bufs=3))
    wpool = ctx.enter_context(tc.tile_pool(name="wpool", bufs=1))
    xpool = ctx.enter_context(tc.tile_pool(name="xpool", bufs=1))
    vpool = ctx.enter_context(tc.tile_pool(name="vpool", bufs=1))
    qkpool = ctx.enter_context(tc.tile_pool(name="qkpool", bufs=2))
    ptpool = ctx.enter_context(tc.tile_pool(name="ptpool", bufs=4))
    osb_pool = ctx.enter_context(tc.tile_pool(name="osb_pool", bufs=2))
    trs_pool = ctx.enter_context(tc.tile_pool(name="trs_pool", bufs=4))
    rspool = ctx.enter_context(tc.tile_pool(name="rspool", bufs=4))
    outpool = ctx.enter_context(tc.tile_pool(name="outpool", bufs=2))

    psum_misc = ctx.enter_context(tc.tile_pool(name="psum_misc", bufs=2, space="PSUM"))
    psum_sring = ctx.enter_context(tc.tile_pool(name="psum_sring", bufs=2, space="PSUM"))
    psum_o = ctx.enter_context(tc.tile_pool(name="psum_o", bufs=2, space="PSUM"))

    # ---------------- constants ----------------
    identity = consts.tile([128, 128], F32)
    nc.sync.dma_start(out=identity, in_=ident)

    bq_sb = consts.tile([128, NPAIR], F32)
    bk_sb = consts.tile([128, NPAIR], F32)
    for p in range(NPAIR):
        nc.sync.dma_start(out=bq_sb[:, p : p + 1], in_=bq[p * 128 : (p + 1) * 128])
        nc.sync.dma_start(out=bk_sb[:, p : p + 1], in_=bk[p * 128 : (p + 1) * 128])
    bv_bcast = consts.tile([128, COUT], F32)
    nc.sync.dma_start(
        out=bv_bcast,
        in_=bass.AP(tensor=bv.tensor, offset=bv.offset, ap=[[0, 128]] + bv.ap),
    )

    # ---------------- transposed fp16 loads: PE transpose + cast-copies -------
    # dst layout [128 (c_in within chunk), KC, ncols]; copies alternate DVE/ACT.
    copy_flip = [0]

    def transpose_rowchunk(dst, src, r, nm):
        cb = cb_pool.tile([128, CIN], F32, tag="cb", name=f"cb_{nm}_{r}")
        nc.sync.dma_start(out=cb, in_=src[r * 128 : (r + 1) * 128, :])
        for g in range(KC // 4):
            trg = psum_misc.tile([128, 4, 128], F32, tag="proj",
                                 name=f"trg_{nm}_{r}_{g}")
            for k4 in range(4):
                kc = g * 4 + k4
                nc.tensor.transpose(
                    trg[:, k4, :], cb[:, kc * 128 : (kc + 1) * 128], identity
                )
            dst_sl = dst[:, g * 4 : (g + 1) * 4, r * 128 : (r + 1) * 128]
            if copy_flip[0] % 2 == 0:
                nc.vector.tensor_copy(dst_sl, trg)
            else:
                nc.scalar.copy(dst_sl, trg)
            copy_flip[0] += 1

    wq_t = wpool.tile([128, KC, COUT], mm_dt)
    wk_t = wpool.tile([128, KC, COUT], mm_dt)
    wv_t = wpool.tile([128, KC, COUT], mm_dt)
    x_t = xpool.tile([128, KC, T], mm_dt)
    for wt, wsrc, wname in ((wq_t, wq, "wq"), (wk_t, wk, "wk"), (wv_t, wv, "wv")):
        for r in range(COUT // 128):
            transpose_rowchunk(wt, wsrc, r, wname)

    # ---------------- projections ----------------
    proj_ps = {}

    def qk_proj_group(p, dst, wt, b_sb, nm, nb, half):
        """Half a projection column-group: 4 contraction chunks; the closing
        half adds the bias and writes fp16 SBUF."""
        ps_name = f"ps_{nm}_{p}_{nb}"
        if half == 0:
            ps = psum_misc.tile([128, 512], F32, tag="proj", name=ps_name)
            proj_ps[(nm, p, nb)] = ps
        else:
            ps = proj_ps.pop((nm, p, nb))
        for k4 in range(4):
            kc = half * 4 + k4
            nc.tensor.matmul(
                ps,
                wt[:, kc, p * 128 : (p + 1) * 128],
                x_t[:, kc, nb * 512 : (nb + 1) * 512],
                start=(kc == 0),
                stop=(kc == KC - 1),
            )
        if half == 1:
            nc.vector.tensor_scalar_add(
                dst[:, nb * 512 : (nb + 1) * 512], ps, b_sb[:, p : p + 1]
            )

    # v_aug [128 (s within chunk), SC, 8 heads, 65]; col 64 == 1.0
    v_aug = vpool.tile([128, SC, 8, 65], mm_dt)
    nc.vector.memset(v_aug[:, :, :, 64:65], 1.0)

    def v_chunk_half(m, half):
        if half == 0:
            ps = psum_misc.tile([128, COUT], F32, tag="proj", name=f"psv_{m}")
            proj_ps[("v", m)] = ps
        else:
            ps = proj_ps.pop(("v", m))
        for k4 in range(4):
            kc = half * 4 + k4
            nc.tensor.matmul(
                ps,
                x_t[:, kc, m * 128 : (m + 1) * 128],
                wv_t[:, kc, :],
                start=(kc == 0),
                stop=(kc == KC - 1),
            )
        if half == 1:
            nc.vector.tensor_add(
                v_aug[:, m, :, 0:64],
                ps.rearrange("p (h d) -> p h d", h=8),
                bv_bcast.rearrange("p (h d) -> p h d", h=8),
            )

    # interleave x transposes with pair-0 projection groups so the first scores
    # panel is ready as soon as possible
    q0 = qkpool.tile([128, T], mm_dt, tag="q", name="qT_0")
    k0 = qkpool.tile([128, T], mm_dt, tag="k", name="kT_0")
    for nb in range(NB):
        for m in range(nb * 4, nb * 4 + 4):
            transpose_rowchunk(x_t, x, m, "x")
        for half in range(2):
            qk_proj_group(0, q0, wq_t, bq_sb, "q", nb, half)
        for half in range(2):
            qk_proj_group(0, k0, wk_t, bk_sb, "k", nb, half)
    qk_tiles = {0: (q0, k0)}

    # background work queue, pumped into the attention loop's PE slack
    bg = []

    def pump(n):
        for _ in range(n):
            if bg:
                bg.pop(0)()

    # ---------------- per head-pair attention ----------------
    def attention_pair(p, q_t, k_t, interleave_v=False):
        out_stage = outpool.tile([128, SC, 128], F32, tag="ostage", name=f"ostage_{p}")
        for tb in range(NB):
            o_ps = [
                psum_o.tile([65, 512], F32, tag="o", name=f"o_{p}_{tb}_{h}")
                for h in range(2)
            ]
            for sj in range(SC):
                sl = psum_sring.tile([128, 2, 512], F32, tag="s", name=f"sl_{p}_{tb}_{sj}")
                for h in range(2):
                    nc.tensor.matmul(
                        sl[:, h, :],
                        k_t[h * 64 : h * 64 + 64, sj * 128 : (sj + 1) * 128],
                        q_t[h * 64 : h * 64 + 64, tb * 512 : (tb + 1) * 512],
                        start=True,
                        stop=True,
                        tile_position=(h * 64, 0) if packed_scores else None,
                    )
                pt = ptpool.tile([128, 2, 512], mm_dt, tag="pt", name=f"pt_{p}_{tb}_{sj}")
                nc.scalar.activation(pt, sl, mybir.ActivationFunctionType.Exp, scale=0.125)
                if interleave_v and tb == 0:
                    # close chunk sj (it is consumed right below), open chunk sj+1
                    v_chunk_half(sj, 1)
                    if sj + 1 < SC:
                        v_chunk_half(sj + 1, 0)
                elif tb >= 2:
                    pump(1)
                for h in range(2):
                    nc.tensor.matmul(
                        o_ps[h],
                        v_aug[:, sj, 2 * p + h, :],
                        pt[:, h, :],
                        start=(sj == 0),
                        stop=(sj == SC - 1),
                        skip_group_check=True,
                    )
            # epilogue: fp16 copy, xbar transpose to natural layout, normalize
            for h in range(2):
                o_sb = osb_pool.tile([80, 512], mm_dt, tag="osb", name=f"osb_{p}_{tb}_{h}")
                nc.gpsimd.memset(o_sb[64:80, :], 0.0)
                nc.vector.tensor_copy(o_sb[0:65, :], o_ps[h])
                for j in range(4):
                    tr_sb = trs_pool.tile([128, 80], mm_dt, tag="trs",
                                          name=f"trs_{p}_{tb}_{h}_{j}")
                    nc.sync.dma_start(
                        out=tr_sb, in_=o_sb[:, j * 128 : (j + 1) * 128], transpose=True
                    )
                    rs = rspool.tile([128, 1], F32, tag="rs", name=f"rs_{p}_{tb}_{h}_{j}")
                    nc.vector.reciprocal(rs, tr_sb[:, 64:65])
                    nc.vector.tensor_scalar_mul(
                        out_stage[:, tb * 4 + j, h * 64 : (h + 1) * 64],
                        tr_sb[:, 0:64],
                        rs,
                    )
            for j in range(tb * 4, tb * 4 + 4):
                nc.sync.dma_start(
                    out=out_ap[j * 128 : (j + 1) * 128, p * 128 : (p + 1) * 128],
                    in_=out_stage[:, j, :],
                )
        while bg:
            bg.pop(0)()

    # open V chunk 0 before the attention loop (its closing half lands in sj=0)
    v_chunk_half(0, 0)

    for p in range(NPAIR):
        if p + 1 < NPAIR:
            qn = qkpool.tile([128, T], mm_dt, tag="q", name=f"qT_{p+1}")
            kn = qkpool.tile([128, T], mm_dt, tag="k", name=f"kT_{p+1}")
            qk_tiles[p + 1] = (qn, kn)
            for nb in range(NB):
                for half in range(2):
                    bg.append(lambda nb=nb, half=half, qn=qn, p=p: qk_proj_group(
                        p + 1, qn, wq_t, bq_sb, "q", nb, half))
                    bg.append(lambda nb=nb, half=half, kn=kn, p=p: qk_proj_group(
                        p + 1, kn, wk_t, bk_sb, "k", nb, half))
        attention_pair(p, *qk_tiles[p], interleave_v=(p == 0))


def build_nc(T: int = T, mm_dt=F16, packed_scores: bool = True, num_devices: int = N_CORES):
    nc = bacc.Bacc("TRN2", target_bir_lowering=False, debug=False, num_devices=num_devices)
    ins = {
        "x": nc.dram_tensor("x", [T, CIN], F32, kind="ExternalInput").ap(),
        "wq": nc.dram_tensor("wq", [COUT, CIN], F32, kind="ExternalInput").ap(),
        "wk": nc.dram_tensor("wk", [COUT, CIN], F32, kind="ExternalInput").ap(),
        "wv": nc.dram_tensor("wv", [COUT, CIN], F32, kind="ExternalInput").ap(),
        "bq": nc.dram_tensor("bq", [COUT], F32, kind="ExternalInput").ap(),
        "bk": nc.dram_tensor("bk", [COUT], F32, kind="ExternalInput").ap(),
        "bv": nc.dram_tensor("bv", [COUT], F32, kind="ExternalInput").ap(),
        "ident": nc.dram_tensor("ident", [128, 128], F32, kind="ExternalInput").ap(),
    }
    out_ap = nc.dram_tensor("out", [T, COUT], F32, kind="ExternalOutput").ap()
    with tile.TileContext(nc) as tc:
        with ExitStack() as ctx:
            emit(ctx, tc, out_ap, ins, T=T, mm_dt=mm_dt, packed_scores=packed_scores)
    nc.compile()
    return nc


_NC = None
_IDENT = np.eye(128, dtype=np.float32)


def _get_nc():
    global _NC
    if _NC is None:
        _NC = build_nc()
    return _NC


def _make_in_maps(q_x, Wq, bq, Wk, bk, Wv, bv):
    f32 = lambda a: np.ascontiguousarray(np.asarray(a, dtype=np.float32))
    q_x, Wq, bq, Wk, bk, Wv, bv = map(f32, (q_x, Wq, bq, Wk, bk, Wv, bv))
    in_maps = []
    for c in range(N_CORES):
        b, hg = divmod(c, 2)
        sl = slice(hg * COUT, (hg + 1) * COUT)
        in_maps.append({
            "x": q_x[b],
            "wq": np.ascontiguousarray(Wq[sl]),
            "wk": np.ascontiguousarray(Wk[sl]),
            "wv": np.ascontiguousarray(Wv[sl]),
            "bq": np.ascontiguousarray(bq[sl]),
            "bk": np.ascontiguousarray(bk[sl]),
            "bv": np.ascontiguousarray(bv[sl]),
            "ident": _IDENT,
        })
    return in_maps


def kernel(q_x, Wq, bq, Wk, bk, Wv, bv):
    nc = _get_nc()
    in_maps = _make_in_maps(q_x, Wq, bq, Wk, bk, Wv, bv)
    res = bass_utils.run_bass_kernel_spmd(nc, in_maps, core_ids=list(range(N_CORES)))
    out = np.empty((B, T, CIN), np.float32)
    for c in range(N_CORES):
        b, hg = divmod(c, 2)
        out[b, :, hg * COUT : (hg + 1) * COUT] = res.results[c]["out"]
    return out


# revision 18
# speedup vs baseline: 1.0667x; 1.0045x over previous
"""Fused multi-head self-attention (B=4, T=2048, C=1024, H=16) for 8 TRN2 NeuronCores.

Sharding: core c = (b, hg) with b = c // 2 (batch), hg = c % 2 (head group of 8
heads).  Each core computes its batch's QKV projection restricted to its 8 heads
(tensor-parallel split of the weight output dim) and the full attention for those
(batch, head) pairs.  No cross-core communication; outputs are disjoint slices of
the final [B, T, C] tensor.

Per-core kernel (see emit()):
  - x and W are loaded natural (contiguous DMA), transposed on the PE (fp32
    transpose tiles into PSUM) and cast-copied to fp16 SBUF alternately by the
    Vector and Scalar engines — an engine-only path with no DMA ordering hazards.
  - fp16 operands run the PE at 1 elem/cycle/lane; accumulation stays fp32 PSUM.
  - qT/kT per head-pair [128 (2 heads x 64 dims), T] = W_pair @ xT; pair-0's
    projection groups are interleaved with the x-transpose stream and later
    pairs' projections are pumped into the PE slack of the previous pair's
    attention loop.
  - V stored per (s_chunk, head) as v_aug [128, SC, 8, 65] (fp16) with a ones
    65th column so the P @ V_aug matmul also emits the softmax row-sums; V
    projection chunks are split in half and interleaved into pair-0's first
    t-block s-loop.
  - scores^T panels [s 128, 2 heads, t 512] = kT_chunk.T @ qT with the two heads
    packed in the PE array via row tiling (rows 0-63 / 64-127, concurrent).
  - exp on ScalarE (1/sqrt(64) folded into the activation scale), PSUM -> SBUF
    fp16, one N=1024 call per head-pair panel.
  - O_aug^T [65, 512] += v_aug_chunk.T @ P^T accumulated over s-chunks in PSUM.
  - epilogue: PSUM -> SBUF fp16 copy, xbar DMA-transpose (SBUF->SBUF) back to the
    natural [t, d] layout, reciprocal + per-partition scale on VectorE,
    contiguous DMA out per t-block.
"""

from contextlib import ExitStack

import numpy as np

import concourse.bass as bass
import concourse.bacc as bacc
import concourse.tile as tile
from concourse import mybir
from concourse import bass_utils

F32 = mybir.dt.float32
F16 = mybir.dt.float16

B = 4
T = 2048
CIN = 1024
COUT = 512               # per-core output channels (8 heads x 64)
KC = CIN // 128          # contraction chunks
NPAIR = 4                # head pairs per core
D = 64
N_CORES = 8


def emit(ctx: ExitStack, tc: tile.TileContext, out_ap: bass.AP, ins: dict,
         T: int = T, mm_dt=F16, packed_scores: bool = True):
    nc = tc.nc
    x, wq, wk, wv, bq, bk, bv, ident = (
        ins[k] for k in ("x", "wq", "wk", "wv", "bq", "bk", "bv", "ident")
    )
    SC = T // 128            # s-chunks (also t-chunks)
    NB = T // 512            # 512-wide column blocks (proj N-chunks and t-blocks)

    # ---------------- pools ----------------
    consts = ctx.enter_context(tc.tile_pool(name="consts", bufs=1))
    cb_pool = ctx.enter_context(tc.tile_pool(name="cb_pool", bufs=3))
    wpool = ctx.enter_context(tc.tile_pool(name="wpool", bufs=1))
    xpool = ctx.enter_context(tc.tile_pool(name="xpool", bufs=1))
    vpool = ctx.enter_context(tc.tile_pool(name="vpool", bufs=1))
    qkpool = ctx.enter_context(tc.tile_pool(name="qkpool", bufs=2))
    ptpool = ctx.enter_context(tc.tile_pool(name="ptpool", bufs=4))
    osb_pool = ctx.enter_context(tc.tile_pool(name="osb_pool", bufs=2))
    trs_pool = ctx.enter_context(tc.tile_pool(name="trs_pool", bufs=4))
    rspool = ctx.enter_context(tc.tile_pool(name="rspool", bufs=4))
    outpool = ctx.enter_context(tc.tile_pool(name="outpool", bufs=2))

    psum_misc = ctx.enter_context(tc.tile_pool(name="psum_misc", bufs=2, space="PSUM"))
    psum_sring = ctx.enter_context(tc.tile_pool(name="psum_sring", bufs=2, space="PSUM"))
    psum_o = ctx.enter_context(tc.tile_pool(name="psum_o", bufs=2, space="PSUM"))

    # ---------------- constants ----------------
    identity = consts.tile([128, 128], F32)
    nc.sync.dma_start(out=identity, in_=ident)

    bq_sb = consts.tile([128, NPAIR], F32)
    bk_sb = consts.tile([128, NPAIR], F32)
    for p in range(NPAIR):
        nc.sync.dma_start(out=bq_sb[:, p : p + 1], in_=bq[p * 128 : (p + 1) * 128])
        nc.sync.dma_start(out=bk_sb[:, p : p + 1], in_=bk[p * 128 : (p + 1) * 128])
    bv_bcast = consts.tile([128, COUT], F32)
    nc.sync.dma_start(
        out=bv_bcast,
        in_=bass.AP(tensor=bv.tensor, offset=bv.offset, ap=[[0, 128]] + bv.ap),
    )

    # ---------------- transposed fp16 loads: PE transpose + cast-copies -------
    # dst layout [128 (c_in within chunk), KC, ncols]; copies alternate DVE/ACT.
    copy_flip = [0]

    def transpose_rowchunk(dst, src, r, nm):
        cb = cb_pool.tile([128, CIN], F32, tag="cb", name=f"cb_{nm}_{r}")
        nc.sync.dma_start(out=cb, in_=src[r * 128 : (r + 1) * 128, :])
        for g in range(KC // 4):
            trg = psum_misc.tile([128, 4, 128], F32, tag="proj",
                                 name=f"trg_{nm}_{r}_{g}")
            for k4 in range(4):
                kc = g * 4 + k4
                nc.tensor.transpose(
                    trg[:, k4, :], cb[:, kc * 128 : (kc + 1) * 128], identity
                )
            dst_sl = dst[:, g * 4 : (g + 1) * 4, r * 128 : (r + 1) * 128]
            if copy_flip[0] % 2 == 0:
                nc.vector.tensor_copy(dst_sl, trg)
            else:
                nc.scalar.copy(dst_sl, trg)
            copy_flip[0] += 1

    wq_t = wpool.tile([128, KC, COUT], mm_dt)
    wk_t = wpool.tile([128, KC, COUT], mm_dt)
    wv_t = wpool.tile([128, KC, COUT], mm_dt)
    x_t = xpool.tile([128, KC, T], mm_dt)
    for wt, wsrc, wname in ((wq_t, wq, "wq"), (wk_t, wk, "wk"), (wv_t, wv, "wv")):
        for r in range(COUT // 128):
            transpose_rowchunk(wt, wsrc, r, wname)

    # ---------------- projections ----------------
    proj_ps = {}

    def qk_proj_group(p, dst, wt, b_sb, nm, nb, quarter):
        """Quarter of a projection column-group: 2 contraction chunks; the
        closing quarter adds the bias and writes fp16 SBUF."""
        if quarter == 0:
            ps = psum_misc.tile([128, 512], F32, tag="proj", name=f"ps_{nm}_{p}_{nb}")
            proj_ps[(nm, p, nb)] = ps
        ps = proj_ps[(nm, p, nb)]
        for k2 in range(2):
            kc = quarter * 2 + k2
            nc.tensor.matmul(
                ps,
                wt[:, kc, p * 128 : (p + 1) * 128],
                x_t[:, kc, nb * 512 : (nb + 1) * 512],
                start=(kc == 0),
                stop=(kc == KC - 1),
            )
        if quarter == 3:
            proj_ps.pop((nm, p, nb))
            nc.vector.tensor_scalar_add(
                dst[:, nb * 512 : (nb + 1) * 512], ps, b_sb[:, p : p + 1]
            )

    # v_aug [128 (s within chunk), SC, 8 heads, 65]; col 64 == 1.0
    v_aug = vpool.tile([128, SC, 8, 65], mm_dt)
    nc.vector.memset(v_aug[:, :, :, 64:65], 1.0)

    def v_chunk_half(m, half):
        if half == 0:
            ps = psum_misc.tile([128, COUT], F32, tag="proj", name=f"psv_{m}")
            proj_ps[("v", m)] = ps
        else:
            ps = proj_ps.pop(("v", m))
        for k4 in range(4):
            kc = half * 4 + k4
            nc.tensor.matmul(
                ps,
                x_t[:, kc, m * 128 : (m + 1) * 128],
                wv_t[:, kc, :],
                start=(kc == 0),
                stop=(kc == KC - 1),
            )
        if half == 1:
            nc.vector.tensor_add(
                v_aug[:, m, :, 0:64],
                ps.rearrange("p (h d) -> p h d", h=8),
                bv_bcast.rearrange("p (h d) -> p h d", h=8),
            )

    def scores_exp(p, tb, sj, q_t, k_t):
        sl = psum_sring.tile([128, 2, 512], F32, tag="s", name=f"sl_{p}_{tb}_{sj}")
        for h in range(2):
            nc.tensor.matmul(
                sl[:, h, :],
                k_t[h * 64 : h * 64 + 64, sj * 128 : (sj + 1) * 128],
                q_t[h * 64 : h * 64 + 64, tb * 512 : (tb + 1) * 512],
                start=True,
                stop=True,
                tile_position=(h * 64, 0) if packed_scores else None,
            )
        pt = ptpool.tile([128, 2, 512], mm_dt, tag="pt", name=f"pt_{p}_{tb}_{sj}")
        nc.scalar.activation(pt, sl, mybir.ActivationFunctionType.Exp, scale=0.125)
        return pt

    # interleave x transposes with pair-0 projection groups so the first scores
    # panel is ready as soon as possible
    q0 = qkpool.tile([128, T], mm_dt, tag="q", name="qT_0")
    k0 = qkpool.tile([128, T], mm_dt, tag="k", name="kT_0")
    early = {}
    for nb in range(NB):
        for m in range(nb * 4, nb * 4 + 4):
            transpose_rowchunk(x_t, x, m, "x")
        for quarter in range(4):
            qk_proj_group(0, q0, wq_t, bq_sb, "q", nb, quarter)
        for quarter in range(4):
            qk_proj_group(0, k0, wk_t, bk_sb, "k", nb, quarter)
        if nb == 0:
            early[(0, 0)] = scores_exp(0, 0, 0, q0, k0)
            early[(0, 1)] = scores_exp(0, 0, 1, q0, k0)
    qk_tiles = {0: (q0, k0)}

    # background work queue, pumped into the attention loop's PE slack
    bg = []

    def pump(n):
        for _ in range(n):
            if bg:
                bg.pop(0)()

    # ---------------- per head-pair attention ----------------
    def attention_pair(p, q_t, k_t, interleave_v=False, early_panels=None):
        early_panels = early_panels or {}
        out_stage = outpool.tile([128, SC, 128], F32, tag="ostage", name=f"ostage_{p}")
        for tb in range(NB):
            o_ps = [
                psum_o.tile([65, 512], F32, tag="o", name=f"o_{p}_{tb}_{h}")
                for h in range(2)
            ]
            for sj in range(SC):
                pt = early_panels.pop((tb, sj), None)
                if pt is None:
                    pt = scores_exp(p, tb, sj, q_t, k_t)
                if interleave_v and tb == 0:
                    # close chunk sj (it is consumed right below), open chunk sj+1
                    v_chunk_half(sj, 1)
                    if sj + 1 < SC:
                        v_chunk_half(sj + 1, 0)
                elif tb >= 1:
                    pump(1)
                for h in range(2):
                    nc.tensor.matmul(
                        o_ps[h],
                        v_aug[:, sj, 2 * p + h, :],
                        pt[:, h, :],
                        start=(sj == 0),
                        stop=(sj == SC - 1),
                        skip_group_check=True,
                    )
            # epilogue: fp16 copy, xbar transpose to natural layout, normalize
            for h in range(2):
                o_sb = osb_pool.tile([80, 512], mm_dt, tag="osb", name=f"osb_{p}_{tb}_{h}")
                nc.gpsimd.memset(o_sb[64:80, :], 0.0)
                nc.vector.tensor_copy(o_sb[0:65, :], o_ps[h])
                for j in range(4):
                    tr_sb = trs_pool.tile([128, 80], mm_dt, tag="trs",
                                          name=f"trs_{p}_{tb}_{h}_{j}")
                    nc.sync.dma_start(
                        out=tr_sb, in_=o_sb[:, j * 128 : (j + 1) * 128], transpose=True
                    )
                    rs = rspool.tile([128, 1], F32, tag="rs", name=f"rs_{p}_{tb}_{h}_{j}")
                    nc.vector.reciprocal(rs, tr_sb[:, 64:65])
                    nc.vector.tensor_scalar_mul(
                        out_stage[:, tb * 4 + j, h * 64 : (h + 1) * 64],
                        tr_sb[:, 0:64],
                        rs,
                    )
            for j in range(tb * 4, tb * 4 + 4):
                nc.sync.dma_start(
                    out=out_ap[j * 128 : (j + 1) * 128, p * 128 : (p + 1) * 128],
                    in_=out_stage[:, j, :],
                )
        while bg:
            bg.pop(0)()

    # open V chunk 0 before the attention loop (its closing half lands in sj=0)
    v_chunk_half(0, 0)

    for p in range(NPAIR):
        if p + 1 < NPAIR:
            qn = qkpool.tile([128, T], mm_dt, tag="q", name=f"qT_{p+1}")
            kn = qkpool.tile([128, T], mm_dt, tag="k", name=f"kT_{p+1}")
            qk_tiles[p + 1] = (qn, kn)
            for nb in range(NB):
                for quarter in range(4):
                    bg.append(lambda nb=nb, quarter=quarter, qn=qn, p=p: qk_proj_group(
                        p + 1, qn, wq_t, bq_sb, "q", nb, quarter))
                for quarter in range(4):
                    bg.append(lambda nb=nb, quarter=quarter, kn=kn, p=p: qk_proj_group(
                        p + 1, kn, wk_t, bk_sb, "k", nb, quarter))
        attention_pair(p, *qk_tiles[p], interleave_v=(p == 0),
                       early_panels=(early if p == 0 else None))


def build_nc(T: int = T, mm_dt=F16, packed_scores: bool = True, num_devices: int = N_CORES):
    nc = bacc.Bacc("TRN2", target_bir_lowering=False, debug=False, num_devices=num_devices)
    ins = {
        "x": nc.dram_tensor("x", [T, CIN], F32, kind="ExternalInput").ap(),
        "wq": nc.dram_tensor("wq", [COUT, CIN], F32, kind="ExternalInput").ap(),
        "wk": nc.dram_tensor("wk", [COUT, CIN], F32, kind="ExternalInput").ap(),
        "wv": nc.dram_tensor("wv", [COUT, CIN], F32, kind="ExternalInput").ap(),
        "bq": nc.dram_tensor("bq", [COUT], F32, kind="ExternalInput").ap(),
        "bk": nc.dram_tensor("bk", [COUT], F32, kind="ExternalInput").ap(),
        "bv": nc.dram_tensor("bv", [COUT], F32, kind="ExternalInput").ap(),
        "ident": nc.dram_tensor("ident", [128, 128], F32, kind="ExternalInput").ap(),
    }
    out_ap = nc.dram_tensor("out", [T, COUT], F32, kind="ExternalOutput").ap()
    with tile.TileContext(nc) as tc:
        with ExitStack() as ctx:
            emit(ctx, tc, out_ap, ins, T=T, mm_dt=mm_dt, packed_scores=packed_scores)
    nc.compile()
    return nc


_NC = None
_IDENT = np.eye(128, dtype=np.float32)


def _get_nc():
    global _NC
    if _NC is None:
        _NC = build_nc()
    return _NC


def _make_in_maps(q_x, Wq, bq, Wk, bk, Wv, bv):
    f32 = lambda a: np.ascontiguousarray(np.asarray(a, dtype=np.float32))
    q_x, Wq, bq, Wk, bk, Wv, bv = map(f32, (q_x, Wq, bq, Wk, bk, Wv, bv))
    in_maps = []
    for c in range(N_CORES):
        b, hg = divmod(c, 2)
        sl = slice(hg * COUT, (hg + 1) * COUT)
        in_maps.append({
            "x": q_x[b],
            "wq": np.ascontiguousarray(Wq[sl]),
            "wk": np.ascontiguousarray(Wk[sl]),
            "wv": np.ascontiguousarray(Wv[sl]),
            "bq": np.ascontiguousarray(bq[sl]),
            "bk": np.ascontiguousarray(bk[sl]),
            "bv": np.ascontiguousarray(bv[sl]),
            "ident": _IDENT,
        })
    return in_maps


def kernel(q_x, Wq, bq, Wk, bk, Wv, bv):
    nc = _get_nc()
    in_maps = _make_in_maps(q_x, Wq, bq, Wk, bk, Wv, bv)
    res = bass_utils.run_bass_kernel_spmd(nc, in_maps, core_ids=list(range(N_CORES)))
    out = np.empty((B, T, CIN), np.float32)
    for c in range(N_CORES):
        b, hg = divmod(c, 2)
        out[b, :, hg * COUT : (hg + 1) * COUT] = res.results[c]["out"]
    return out
